# revision 23
# baseline (speedup 1.0000x reference)
"""Trainium2 Bass kernel for nn_Loss_fun_24421184045291.

Loss = BCE(fused) + mean_v BCE(view_v) + sup_contrastive + 0.2 * unsup_consistency.

Math reductions (vs the reference):
  * sup denominator mask == ~eye; pos_count == 3071 for every anchor; all
    anchors valid (these follow from the index structure, not the values).
  * positive-pair sums collapse analytically:
        sup   pt_i = (zf_i . S_label(i) - ||zf_i||^2) / (T * 3071)
        unsup pt_i = (z_i . S_node(i)  - ||z_i||^2) / (T * (V-1))
  * the unsup re-normalization is a no-op: proj rows are unit-norm, so
    zn = zf * (1 +- 1e-7); skip it.
  * both 6144x6144 similarity matrices are symmetric: each unordered block
    pair is computed once via a cyclic cover.  exp row-sums ride the scalar
    engine's accum_out; the transposed contributions are column sums of the
    exp'd blocks (ones-indicator matmuls stacking into PSUM partition rows,
    one DVE copy per run).

Cyclic symmetric cover (48 row-tiles of 128): tile ti owns blocks
(ti, (ti+d) mod 48) for d = 0..23, plus d = 24 when ti < 24.  Every unordered
pair is computed exactly once, and each tile's column window is contiguous
mod 6144.  Core c takes tiles {3c, 3c+1, 3c+2, 24+3c, 25+3c, 26+3c}.  The
host ships each core a column-rotated fp8 table (by -384c, with a 256-col
wrap extension), so the device program is identical on all cores (SPMD) and
all per-core variation is data.  Matmuls run in fp8(e4m3) DoubleRow perf
mode (K=256 per instruction, 2x PE rate).  Each core ships per-row partials
(rowsum/diag columns, colsum vectors, positive-term row-dots, BCE partial
sums); the host assembles denominators, logs, and means in float64.
"""

import sys
from contextlib import ExitStack

import numpy as np

if "/opt/trn_rl_repo" not in sys.path:
    sys.path.insert(0, "/opt/trn_rl_repo")

import concourse.bass as bass
import concourse.tile as tile
from concourse import bacc, mybir
from concourse import bass_utils
from concourse.masks import make_identity

# ---------------------------------------------------------------- constants
TEMP = 0.2
ISC = 1.0 / TEMP
L_MAIN, L_VIEW, L_SUP, L_UNSUP = 1.0, 1.0, 1.0, 0.2
N, D, V, PP, NEG, U = 100000, 256, 3, 1024, 1024, 2048

NCORES = 8
M = (PP + NEG) * V          # 6144 rows/cols of both similarity matrices
P = 128
NT = M // P                 # 48 row tiles
MC = M // NCORES            # 768 pt rows per core
NS = N // NCORES            # 12500 BCE elements per core
W = 98                      # padded BCE free width (128*98 = 12544 >= 12500)
SUP_CNT = float((PP - 1) * V + (V - 1))   # 3071 positives per sup anchor
GRP = 1536                  # psum group width (3 banks)
TW = M + 256                # rotated table width incl. wrap extension (6400)
ROT = 384                   # per-core column rotation step (3 tiles)
RUNS = (0, 1, 2, 24, 25, 26)          # local row tiles per core
RW = (3200, 3200, 3200, 3072, 3072, 3072)   # run widths (incl. 128 diag)

F32 = mybir.dt.float32
BF16 = mybir.dt.bfloat16
FP8 = mybir.dt.float8e4

_PROGRAM_CACHE = {}
import os
KPART = os.environ.get("KPART", "all")   # bce | corr | main | nocs | all
KRUNS = int(os.environ.get("KRUNS", "12"))   # number of (X, rho) runs
KDG = os.environ.get("KDG", "1") == "1"      # diag extract via custom DVE op
KPACK = os.environ.get("KPACK", "1") == "1"  # pack diag+chunks into banks


def run_chunks(rho):
    """Colsum chunks of run rho as (local_col_start, width), diag excluded."""
    base = 128 * RUNS[rho] + 128
    total = RW[rho] - 128
    out = []
    a = 0
    while a < total:
        w = min(512, total - a)
        out.append((base + a, w))
        a += w
    return out


# ---------------------------------------------------------------- device code
def _loss_body(ctx: ExitStack, tc, io):
    nc = tc.nc
    AF = mybir.ActivationFunctionType
    OP = mybir.AluOpType
    AX = mybir.AxisListType

    (stab8, utab8, slhs, ulhs, snode, ssel,
     blog, vlog, blab, bmsk, prd, pcs, pcor, pbce) = io

    sb_tab = ctx.enter_context(tc.tile_pool(name="sb_tab", bufs=1))
    sb_e = ctx.enter_context(tc.tile_pool(name="sb_e", bufs=2))
    sb_sl = ctx.enter_context(tc.tile_pool(name="sb_sl", bufs=1))
    sb_tmp = ctx.enter_context(tc.tile_pool(name="sb_tmp", bufs=2))
    sb_cs = ctx.enter_context(tc.tile_pool(name="sb_cs", bufs=2))
    sb_sm = ctx.enter_context(tc.tile_pool(name="sb_sm", bufs=1))
    sb_bce = ctx.enter_context(tc.tile_pool(name="sb_bce", bufs=2))
    ps_mm = ctx.enter_context(tc.tile_pool(name="ps_mm", bufs=2, space="PSUM"))
    ps_cs = ctx.enter_context(tc.tile_pool(name="ps_cs", bufs=2, space="PSUM"))

    # ---- constants -----------------------------------------------------
    ident = sb_sm.tile([P, P], F32)
    make_identity(nc, ident[:])
    indT = sb_sm.tile([P, 16], BF16)
    nc.vector.memset(indT, 0.0)
    nc.vector.memset(indT[:, 7:8], 1.0)
    ones_b = sb_sm.tile([P, 1], BF16)
    nc.vector.memset(ones_b, 1.0)

    # ---- DMA issues ----------------------------------------------------
    sl, ul, sn, sse = [], [], [], []
    for k in range(2):
        t = sb_sl.tile([P, MC], BF16, name=f"sl{k}", tag=f"sl{k}")
        nc.sync.dma_start(out=t, in_=slhs[k])
        sl.append(t)
        t = sb_sl.tile([P, MC], BF16, name=f"ul{k}", tag=f"ul{k}")
        nc.sync.dma_start(out=t, in_=ulhs[k])
        ul.append(t)
        t = sb_sl.tile([P, MC], BF16, name=f"sn{k}", tag=f"sn{k}")
        nc.sync.dma_start(out=t, in_=snode[k])
        sn.append(t)
        t = sb_sl.tile([P, 1], BF16, name=f"se{k}", tag=f"se{k}")
        nc.sync.dma_start(out=t, in_=ssel[k])
        sse.append(t)

    lab_t = sb_sm.tile([P, W], F32)
    nc.sync.dma_start(out=lab_t, in_=blab)
    msk_t = sb_sm.tile([P, W], F32)
    nc.sync.dma_start(out=msk_t, in_=bmsk)
    bce_x = []
    for i, src_ap in enumerate([blog] + [vlog[v] for v in range(V)]):
        x = sb_bce.tile([P, W], F32, name=f"bce_x{i}", tag=f"bce_x{i}")
        nc.sync.dma_start(out=x, in_=src_ap)
        bce_x.append(x)

    # Stream both tables over BOTH queues (gpsimd + sync) in 8 half-chunks
    # each, sup first: halves the arrival time of the data the early runs
    # starve on.
    tabs = []
    for nm, src in (("st", stab8), ("ut", utab8)):
        tabs.append(sb_tab.tile([P, 2, TW], FP8, name=nm, tag=nm))
    cw = TW // 4
    hw = cw // 2
    for ti, src in ((0, stab8), (1, utab8)):
        t = tabs[ti]
        for g in range(4):
            for h in range(2):
                eng = nc.gpsimd if h == 0 else nc.sync
                lo = g * cw + h * hw
                eng.dma_start(out=t[:, :, lo:lo + hw],
                              in_=src[g, :, :, h * hw:(h + 1) * hw])

    # ---- BCE phase 1 ---------------------------------------------------
    bce_e, bce_pb = [], []
    for i in range(1 + V):
        x = bce_x[i]
        e = sb_sm.tile([P, W], F32, name=f"bce_e{i}", tag=f"bce_e{i}")
        nc.scalar.activation(e, x, AF.Abs)
        nc.scalar.activation(e, e, AF.Exp, scale=-1.0)
        bce_e.append(e)
        pb = sb_sm.tile([P, W], F32, name=f"bce_pb{i}", tag=f"bce_pb{i}")
        nc.scalar.activation(pb, x, AF.Relu)
        xy = sb_bce.tile([P, W], F32, name="bce_xy", tag="bce_xy")
        nc.vector.tensor_mul(xy, x, lab_t)
        nc.vector.tensor_sub(pb, pb, xy)
        bce_pb.append(pb)

    partcols = sb_sm.tile([P, 8], F32)
    nc.vector.memset(partcols, 0.0)
    nc.vector.reduce_sum(out=partcols[:, 4:5], in_=msk_t, axis=AX.X)

    # ---- corrections: qs, ss, qu, ssu ----------------------------------
    def rowdot(vec_tiles, rhs_tiles, tag):
        res = sb_sm.tile([1, MC], F32, name=f"rd_{tag}", tag=f"rd_{tag}")
        for j0 in range(0, MC, 512):
            w = min(512, MC - j0)
            pq = ps_cs.tile([16, 512], F32, name="pq", tag="pscs")
            for k in range(2):
                nc.tensor.matmul(pq[:1, :w], lhsT=vec_tiles[k],
                                 rhs=rhs_tiles[k][:, j0:j0 + w],
                                 start=(k == 0), stop=(k == 1))
            nc.vector.tensor_copy(res[:, j0:j0 + w], pq[:1, :w])
        return res

    sq_s, sq_u, sxx = [], [], []
    for k in range(2):
        q = sb_tmp.tile([P, MC], BF16, name="sq_s", tag=f"sq_s{k}")
        nc.vector.tensor_mul(q, sl[k], sl[k])
        sq_s.append(q)
        q = sb_tmp.tile([P, MC], BF16, name="sq_u", tag=f"sq_u{k}")
        nc.vector.tensor_mul(q, ul[k], ul[k])
        sq_u.append(q)
        q = sb_tmp.tile([P, MC], BF16, name="sx", tag=f"sx{k}")
        nc.vector.tensor_mul(q, ul[k], sn[k])
        sxx.append(q)

    if KPART in ("corr", "main", "nocs", "all"):
        corr_rows = [rowdot(sse, sl, "qs"),
                     rowdot([ones_b, ones_b], sq_s, "ss"),
                     rowdot([ones_b, ones_b], sxx, "qu"),
                     rowdot([ones_b, ones_b], sq_u, "ssu")]
        for r, row in enumerate(corr_rows):
            nc.gpsimd.dma_start(out=pcor[r:r + 1], in_=row)

    # ---- main symmetric cyclic-cover loop ------------------------------
    rdcols = sb_sm.tile([P, 24], F32)
    nc.vector.memset(rdcols, 0.0)
    dg_scr = sb_tmp.tile([P, P], BF16, name="dg_scr", tag="dg_scr")

    def emit_colsums(X, rho, E):
        chunks = run_chunks(rho)
        CS = ps_cs.tile([16, 512], F32, name="cs", tag="pscs")
        for j, (a, w) in enumerate(chunks):
            eo = a - 128 * RUNS[rho]
            nc.tensor.matmul(CS[0:8, 0:w], lhsT=indT[:, 7 - j:15 - j],
                             rhs=E[:, eo:eo + w],
                             start=(j == 0), stop=(j == len(chunks) - 1),
                             skip_group_check=True)
        cst = sb_cs.tile([16, 512], F32, name="cst", tag="cst")
        nch = len(chunks)
        nc.vector.tensor_copy(cst[0:nch], CS[0:nch])
        nc.gpsimd.dma_start(out=pcs[X, rho, 0:nch], in_=cst[0:nch])

    pend = []
    matrices = range(2) if KPART in ("main", "nocs", "all") else range(0)
    nrun = 0
    for X in matrices:
        tab = tabs[X]
        for rho, r in enumerate(RUNS):
            nrun += 1
            if nrun > KRUNS:
                break
            rw = RW[rho]
            base = 128 * r
            lhsT = tab[:, :, base:base + P]
            E = sb_e.tile([P, 3200], BF16, name="E",
                          tag=f"E{(X * 6 + rho) % 2}")
            racc = sb_sm.tile([P, 3], F32, name="racc",
                              tag=f"racc{(X * 6 + rho) % 2}")
            ngrp = (rw + GRP - 1) // GRP
            for gi in range(ngrp):
                glo = gi * GRP
                ghi = min(rw, glo + GRP)
                pg = ps_mm.tile([P, GRP], F32, name="pg", tag="psmm")
                pos = 0
                while pos < ghi - glo:
                    if KPACK:
                        seg = min(512 - pos % 512, ghi - glo - pos)
                        if gi == 0 and pos == 0:
                            seg = P        # diag block first
                    else:
                        seg = min(512, ghi - glo - pos)
                    st = (pos % 512) == 0 or not KPACK
                    last = (pos + seg >= ghi - glo) or \
                        ((pos + seg) % 512 == 0) or not KPACK
                    nc.tensor.matmul(
                        pg[:, pos:pos + seg],
                        lhsT=lhsT,
                        rhs=tab[:, :, base + glo + pos:base + glo + pos + seg],
                        start=st, stop=last,
                        perf_mode=mybir.MatmulPerfMode.DoubleRow,
                        skip_group_check=True)
                    pos += seg
                nc.scalar.activation(E[:, glo:ghi], pg[:, 0:ghi - glo],
                                     AF.Exp, scale=ISC,
                                     accum_out=racc[:, gi:gi + 1])
            nc.vector.reduce_sum(out=rdcols[:, X * 12 + rho:X * 12 + rho + 1],
                                 in_=racc[:, 0:ngrp], axis=AX.X)
            if KDG:
                nc.vector.tensor_mul(dg_scr, E[:, 0:P], ident)
                nc.vector.reduce_sum(
                    out=rdcols[:, X * 12 + 6 + rho:X * 12 + 7 + rho],
                    in_=dg_scr, axis=AX.X)
            if KPART != "nocs":
                pend.append((X, rho, E))
            if len(pend) > 1:
                emit_colsums(*pend.pop(0))
    while pend:
        emit_colsums(*pend.pop(0))

    # ---- BCE phase 2 ---------------------------------------------------
    # The Ln bias comes from a tile that depends on the main loop's last
    # write, pinning these Ln ops after every Exp (one act-table switch
    # instead of a per-run EXP<->LN ping-pong).
    ln_bias = sb_sm.tile([P, 1], F32)
    nc.vector.reduce_max(out=ln_bias, in_=rdcols[:, 15:17], axis=AX.X)
    nc.vector.tensor_scalar_mul(ln_bias, ln_bias, 0.0)
    nc.vector.tensor_scalar_add(ln_bias, ln_bias, 1.0)
    for i in range(1 + V):
        e, pb = bce_e[i], bce_pb[i]
        nc.scalar.activation(e, e, AF.Ln, bias=ln_bias)  # log1p(exp(-|x|))
        nc.vector.tensor_add(pb, pb, e)
        nc.vector.tensor_mul(pb, pb, msk_t)
        nc.vector.reduce_sum(out=partcols[:, i:i + 1], in_=pb, axis=AX.X)

    nc.gpsimd.dma_start(out=prd, in_=rdcols)
    nc.gpsimd.dma_start(out=pbce, in_=partcols)


# ---------------------------------------------------------------- program
def build_program():
    nc = bacc.Bacc("TRN2", target_bir_lowering=False, debug=False,
                   num_devices=NCORES)
    io = (
        nc.dram_tensor("stab8", (4, P, 2, TW // 4), FP8,
                       kind="ExternalInput").ap(),
        nc.dram_tensor("utab8", (4, P, 2, TW // 4), FP8,
                       kind="ExternalInput").ap(),
        nc.dram_tensor("slhs", (2, P, MC), BF16, kind="ExternalInput").ap(),
        nc.dram_tensor("ulhs", (2, P, MC), BF16, kind="ExternalInput").ap(),
        nc.dram_tensor("snode", (2, P, MC), BF16, kind="ExternalInput").ap(),
        nc.dram_tensor("ssel", (2, P, 1), BF16, kind="ExternalInput").ap(),
        nc.dram_tensor("blog", (P, W), F32, kind="ExternalInput").ap(),
        nc.dram_tensor("vlog", (V, P, W), F32, kind="ExternalInput").ap(),
        nc.dram_tensor("blab", (P, W), F32, kind="ExternalInput").ap(),
        nc.dram_tensor("bmsk", (P, W), F32, kind="ExternalInput").ap(),
        nc.dram_tensor("prd", (P, 24), F32, kind="ExternalOutput").ap(),
        nc.dram_tensor("pcs", (2, 6, 6, 512), F32, kind="ExternalOutput").ap(),
        nc.dram_tensor("pcor", (4, MC), F32, kind="ExternalOutput").ap(),
        nc.dram_tensor("pbce", (P, 8), F32, kind="ExternalOutput").ap(),
    )
    with tile.TileContext(nc) as tc:
        with ExitStack() as ctx:
            _loss_body(ctx, tc, io)
    nc.compile()
    return nc


def get_program():
    if "nc" not in _PROGRAM_CACHE:
        _PROGRAM_CACHE["nc"] = build_program()
    return _PROGRAM_CACHE["nc"]


# ---------------------------------------------------------------- host side
def shard_inputs(fused_logit, view_logits, proj, labels, train_mask,
                 train_pos_idx, train_neg_idx, unlabeled_idx):
    import ml_dtypes

    fused_logit = np.asarray(fused_logit, dtype=np.float32)
    view_logits = np.asarray(view_logits, dtype=np.float32)
    proj = np.asarray(proj, dtype=np.float32)
    labels = np.asarray(labels, dtype=np.float32)
    maskf = np.asarray(train_mask).astype(np.float32)

    lab_idx = np.concatenate([np.asarray(train_pos_idx),
                              np.asarray(train_neg_idx)]).astype(np.int64)
    unl_idx = np.asarray(unlabeled_idx).astype(np.int64)

    zf = proj[:, lab_idx, :].transpose(1, 0, 2).reshape(M, D)
    zu = proj[:, unl_idx, :].transpose(1, 0, 2).reshape(M, D)
    zfT8 = zf.T.astype(ml_dtypes.float8_e4m3)       # [256, 6144]
    zuT8 = zu.T.astype(ml_dtypes.float8_e4m3)

    zfT16 = zf.T.astype(ml_dtypes.bfloat16)
    zuT16 = zu.T.astype(ml_dtypes.bfloat16)
    s_lab1 = zf[:PP * V].sum(axis=0)                # [256] label-1 sum
    s_lab0 = zf[PP * V:].sum(axis=0)
    s_node = zu.reshape(U, V, D).sum(axis=1)        # [2048, 256]
    snodeT = np.repeat(s_node, V, axis=0).T.astype(ml_dtypes.bfloat16)

    def rot_tab(z8, c):
        r = np.roll(z8, -ROT * c, axis=1)
        ext = np.concatenate([r, r[:, :TW - M]], axis=1)      # [256, 6400]
        return np.ascontiguousarray(
            ext.reshape(2, P, 4, TW // 4).transpose(2, 1, 0, 3))

    def pack_bce(x):
        out = np.zeros((NCORES, P, W), dtype=np.float32)
        flat = out.reshape(NCORES, P * W)
        flat[:, :NS] = x.reshape(NCORES, NS)
        return out

    blog = pack_bce(fused_logit)
    vlog = np.stack([pack_bce(view_logits[v]) for v in range(V)], axis=1)
    blab = pack_bce(labels)
    bmsk = pack_bce(maskf)

    in_maps = []
    for c in range(NCORES):
        j0 = c * MC
        sse = (s_lab1 if c < NCORES // 2 else s_lab0).astype(
            ml_dtypes.bfloat16).reshape(2, P, 1)
        in_maps.append(dict(
            stab8=rot_tab(zfT8, c),
            utab8=rot_tab(zuT8, c),
            slhs=np.ascontiguousarray(
                zfT16[:, j0:j0 + MC]).reshape(2, P, MC),
            ulhs=np.ascontiguousarray(
                zuT16[:, j0:j0 + MC]).reshape(2, P, MC),
            snode=np.ascontiguousarray(
                snodeT[:, j0:j0 + MC]).reshape(2, P, MC),
            ssel=sse,
            blog=blog[c],
            vlog=vlog[c],
            blab=blab[c],
            bmsk=bmsk[c],
        ))
    return in_maps


def combine_partials(results):
    """results: per-core dicts with prd/pcs/pcor/pbce -> final (5,) losses."""
    den = np.zeros((2, M), dtype=np.float64)
    dgv = np.zeros((2, M), dtype=np.float64)
    for c, res in enumerate(results):
        prd = res["prd"].astype(np.float64)           # [128, 24]
        pcs = res["pcs"].astype(np.float64)           # [2, 6, 6, 512]
        for X in range(2):
            for rho, r in enumerate(RUNS):
                gt = (r + 3 * c) % NT
                rows = slice(P * gt, P * gt + P)
                den[X][rows] += prd[:, X * 12 + rho]
                dgv[X][rows] += prd[:, X * 12 + 6 + rho]
                for ci, (a, w) in enumerate(run_chunks(rho)):
                    cols = (np.arange(a, a + w) + ROT * c) % M
                    den[X][cols] += pcs[X, rho, ci, :w]
    den = den - dgv + 1e-12

    qs, ss, qu, ssu = np.concatenate(
        [r["pcor"].astype(np.float64) for r in results], axis=1)
    pt_s = (qs - ss) / (TEMP * SUP_CNT)
    pt_u = (qu - ssu) / (TEMP * (V - 1))
    sup = float(np.mean(np.log(den[0]) - pt_s))
    unsup = float(np.mean(np.log(den[1]) - pt_u))

    pb = np.stack([r["pbce"] for r in results]).astype(np.float64)
    tot = pb.sum(axis=(0, 1))                         # [8]
    cnt = max(tot[4], 1.0)
    main = tot[0] / cnt
    view = (tot[1] + tot[2] + tot[3]) / (V * cnt)
    total = L_MAIN * main + L_VIEW * view + L_SUP * sup + L_UNSUP * unsup
    return np.array([total, main, view, sup, unsup], dtype=np.float32)


def kernel(**inputs) -> np.ndarray:
    in_maps = shard_inputs(**inputs)
    nc = get_program()
    res = bass_utils.run_bass_kernel_spmd(nc, in_maps,
                                          core_ids=list(range(NCORES)))
    return combine_partials(res.results)


# revision 25
# speedup vs baseline: 1.0422x; 1.0422x over previous
"""Trainium2 Bass kernel for nn_Loss_fun_24421184045291.

Loss = BCE(fused) + mean_v BCE(view_v) + sup_contrastive + 0.2 * unsup_consistency.

Math reductions (vs the reference):
  * sup denominator mask == ~eye; pos_count == 3071 for every anchor; all
    anchors valid (these follow from the index structure, not the values).
  * positive-pair sums collapse analytically:
        sup   pt_i = (zf_i . S_label(i) - ||zf_i||^2) / (T * 3071)
        unsup pt_i = (z_i . S_node(i)  - ||z_i||^2) / (T * (V-1))
  * the unsup re-normalization is a no-op: proj rows are unit-norm, so
    zn = zf * (1 +- 1e-7); skip it.
  * both 6144x6144 similarity matrices are symmetric: each unordered block
    pair is computed once via a cyclic cover.  exp row-sums ride the scalar
    engine's accum_out; the transposed contributions are column sums of the
    exp'd blocks (ones-indicator matmuls stacking into PSUM partition rows,
    one DVE copy per run).

Cyclic symmetric cover (48 row-tiles of 128): tile ti owns blocks
(ti, (ti+d) mod 48) for d = 0..23, plus d = 24 when ti < 24.  Every unordered
pair is computed exactly once, and each tile's column window is contiguous
mod 6144.  Core c takes tiles {3c, 3c+1, 3c+2, 24+3c, 25+3c, 26+3c}.  The
host ships each core a column-rotated fp8 table (by -384c, with a 256-col
wrap extension), so the device program is identical on all cores (SPMD) and
all per-core variation is data.  Matmuls run in fp8(e4m3) DoubleRow perf
mode (K=256 per instruction, 2x PE rate).  Each core ships per-row partials
(rowsum/diag columns, colsum vectors, positive-term row-dots, BCE partial
sums); the host assembles denominators, logs, and means in float64.
"""

import sys
from contextlib import ExitStack

import numpy as np

if "/opt/trn_rl_repo" not in sys.path:
    sys.path.insert(0, "/opt/trn_rl_repo")

import concourse.bass as bass
import concourse.tile as tile
from concourse import bacc, mybir
from concourse import bass_utils
from concourse.masks import make_identity

# ---------------------------------------------------------------- constants
TEMP = 0.2
ISC = 1.0 / TEMP
L_MAIN, L_VIEW, L_SUP, L_UNSUP = 1.0, 1.0, 1.0, 0.2
N, D, V, PP, NEG, U = 100000, 256, 3, 1024, 1024, 2048

NCORES = 8
M = (PP + NEG) * V          # 6144 rows/cols of both similarity matrices
P = 128
NT = M // P                 # 48 row tiles
MC = M // NCORES            # 768 pt rows per core
NS = N // NCORES            # 12500 BCE elements per core
W = 98                      # padded BCE free width (128*98 = 12544 >= 12500)
SUP_CNT = float((PP - 1) * V + (V - 1))   # 3071 positives per sup anchor
GRP = 1536                  # psum group width (3 banks)
TW = M + 256                # rotated table width incl. wrap extension (6400)
ROT = 384                   # per-core column rotation step (3 tiles)
RUNS = (0, 1, 2, 24, 25, 26)          # local row tiles per core
RW = (3200, 3200, 3200, 3072, 3072, 3072)   # run widths (incl. 128 diag)

F32 = mybir.dt.float32
BF16 = mybir.dt.bfloat16
FP8 = mybir.dt.float8e4

_PROGRAM_CACHE = {}
import os
KPART = os.environ.get("KPART", "all")   # bce | corr | main | nocs | all
KRUNS = int(os.environ.get("KRUNS", "12"))   # number of (X, rho) runs
KDG = os.environ.get("KDG", "1") == "1"      # diag extract via custom DVE op
KPACK = os.environ.get("KPACK", "1") == "1"  # pack diag+chunks into banks


def run_chunks(rho):
    """Colsum chunks of run rho as (local_col_start, width), diag excluded."""
    base = 128 * RUNS[rho] + 128
    total = RW[rho] - 128
    out = []
    a = 0
    while a < total:
        w = min(512, total - a)
        out.append((base + a, w))
        a += w
    return out


# ---------------------------------------------------------------- device code
def _loss_body(ctx: ExitStack, tc, io):
    nc = tc.nc
    AF = mybir.ActivationFunctionType
    OP = mybir.AluOpType
    AX = mybir.AxisListType

    (stab8, utab8, slhs, ulhs, snode, ssel,
     blog, vlog, blab, bmsk, prd, pcs, pcor, pbce) = io

    sb_tab = ctx.enter_context(tc.tile_pool(name="sb_tab", bufs=1))
    sb_e = ctx.enter_context(tc.tile_pool(name="sb_e", bufs=2))
    sb_sl = ctx.enter_context(tc.tile_pool(name="sb_sl", bufs=1))
    sb_tmp = ctx.enter_context(tc.tile_pool(name="sb_tmp", bufs=2))
    sb_cs = ctx.enter_context(tc.tile_pool(name="sb_cs", bufs=2))
    sb_sm = ctx.enter_context(tc.tile_pool(name="sb_sm", bufs=1))
    sb_bce = ctx.enter_context(tc.tile_pool(name="sb_bce", bufs=2))
    ps_mm = ctx.enter_context(tc.tile_pool(name="ps_mm", bufs=2, space="PSUM"))
    ps_cs = ctx.enter_context(tc.tile_pool(name="ps_cs", bufs=2, space="PSUM"))

    # ---- constants -----------------------------------------------------
    ident = sb_sm.tile([P, P], F32)
    make_identity(nc, ident[:])
    indT = sb_sm.tile([P, 16], BF16)
    nc.vector.memset(indT, 0.0)
    nc.vector.memset(indT[:, 7:8], 1.0)
    ones_b = sb_sm.tile([P, 1], BF16)
    nc.vector.memset(ones_b, 1.0)

    # ---- DMA issues ----------------------------------------------------
    # Both tables stream over both queues (8 half-chunks each, sup first).
    # The first two sup half-chunks outrank the small inputs on sync so the
    # first runs don't starve.
    tabs = []
    for nm in ("st", "ut"):
        tabs.append(sb_tab.tile([P, 2, TW], FP8, name=nm, tag=nm))
    cw = TW // 4
    hw = cw // 2

    def tab_dma(ti, g, h, eng):
        src = (stab8, utab8)[ti]
        lo = g * cw + h * hw
        eng.dma_start(out=tabs[ti][:, :, lo:lo + hw],
                      in_=src[g, :, :, h * hw:(h + 1) * hw])

    for g in range(4):
        tab_dma(0, g, 0, nc.gpsimd)
    tab_dma(0, 0, 1, nc.sync)
    tab_dma(0, 1, 1, nc.sync)

    sl, ul, sn, sse = [], [], [], []
    for k in range(2):
        t = sb_sl.tile([P, MC], BF16, name=f"sl{k}", tag=f"sl{k}")
        nc.sync.dma_start(out=t, in_=slhs[k])
        sl.append(t)
        t = sb_sl.tile([P, MC], BF16, name=f"ul{k}", tag=f"ul{k}")
        nc.sync.dma_start(out=t, in_=ulhs[k])
        ul.append(t)
        t = sb_sl.tile([P, MC], BF16, name=f"sn{k}", tag=f"sn{k}")
        nc.sync.dma_start(out=t, in_=snode[k])
        sn.append(t)
        t = sb_sl.tile([P, 1], BF16, name=f"se{k}", tag=f"se{k}")
        nc.sync.dma_start(out=t, in_=ssel[k])
        sse.append(t)

    lab_t = sb_sm.tile([P, W], F32)
    nc.sync.dma_start(out=lab_t, in_=blab)
    msk_t = sb_sm.tile([P, W], F32)
    nc.sync.dma_start(out=msk_t, in_=bmsk)
    bce_x = []
    for i, src_ap in enumerate([blog] + [vlog[v] for v in range(V)]):
        x = sb_bce.tile([P, W], F32, name=f"bce_x{i}", tag=f"bce_x{i}")
        nc.sync.dma_start(out=x, in_=src_ap)
        bce_x.append(x)

    # remaining sup halves, then the unsup table split over both queues
    tab_dma(0, 2, 1, nc.sync)
    tab_dma(0, 3, 1, nc.sync)
    for g in range(4):
        tab_dma(1, g, 0, nc.gpsimd)
        tab_dma(1, g, 1, nc.sync)

    # ---- BCE phase 1 ---------------------------------------------------
    bce_e, bce_pb = [], []
    for i in range(1 + V):
        x = bce_x[i]
        e = sb_sm.tile([P, W], F32, name=f"bce_e{i}", tag=f"bce_e{i}")
        nc.scalar.activation(e, x, AF.Abs)
        nc.scalar.activation(e, e, AF.Exp, scale=-1.0)
        bce_e.append(e)
        pb = sb_sm.tile([P, W], F32, name=f"bce_pb{i}", tag=f"bce_pb{i}")
        nc.scalar.activation(pb, x, AF.Relu)
        xy = sb_bce.tile([P, W], F32, name="bce_xy", tag="bce_xy")
        nc.vector.tensor_mul(xy, x, lab_t)
        nc.vector.tensor_sub(pb, pb, xy)
        bce_pb.append(pb)

    partcols = sb_sm.tile([P, 8], F32)
    nc.vector.memset(partcols, 0.0)
    nc.vector.reduce_sum(out=partcols[:, 4:5], in_=msk_t, axis=AX.X)

    # ---- corrections: qs, ss, qu, ssu ----------------------------------
    def rowdot(vec_tiles, rhs_tiles, tag):
        res = sb_sm.tile([1, MC], F32, name=f"rd_{tag}", tag=f"rd_{tag}")
        for j0 in range(0, MC, 512):
            w = min(512, MC - j0)
            pq = ps_cs.tile([16, 512], F32, name="pq", tag="pscs")
            for k in range(2):
                nc.tensor.matmul(pq[:1, :w], lhsT=vec_tiles[k],
                                 rhs=rhs_tiles[k][:, j0:j0 + w],
                                 start=(k == 0), stop=(k == 1))
            nc.vector.tensor_copy(res[:, j0:j0 + w], pq[:1, :w])
        return res

    sq_s, sq_u, sxx = [], [], []
    for k in range(2):
        q = sb_tmp.tile([P, MC], BF16, name="sq_s", tag=f"sq_s{k}")
        nc.vector.tensor_mul(q, sl[k], sl[k])
        sq_s.append(q)
        q = sb_tmp.tile([P, MC], BF16, name="sq_u", tag=f"sq_u{k}")
        nc.vector.tensor_mul(q, ul[k], ul[k])
        sq_u.append(q)
        q = sb_tmp.tile([P, MC], BF16, name="sx", tag=f"sx{k}")
        nc.vector.tensor_mul(q, ul[k], sn[k])
        sxx.append(q)

    if KPART in ("corr", "main", "nocs", "all"):
        corr_rows = [rowdot(sse, sl, "qs"),
                     rowdot([ones_b, ones_b], sq_s, "ss"),
                     rowdot([ones_b, ones_b], sxx, "qu"),
                     rowdot([ones_b, ones_b], sq_u, "ssu")]
        for r, row in enumerate(corr_rows):
            nc.gpsimd.dma_start(out=pcor[r:r + 1], in_=row)

    # ---- main symmetric cyclic-cover loop ------------------------------
    rdcols = sb_sm.tile([P, 24], F32)
    nc.vector.memset(rdcols, 0.0)
    dg_scr = sb_tmp.tile([P, P], BF16, name="dg_scr", tag="dg_scr")

    def emit_colsums(X, rho, E):
        chunks = run_chunks(rho)
        CS = ps_cs.tile([16, 512], F32, name="cs", tag="pscs")
        for j, (a, w) in enumerate(chunks):
            eo = a - 128 * RUNS[rho]
            nc.tensor.matmul(CS[0:8, 0:w], lhsT=indT[:, 7 - j:15 - j],
                             rhs=E[:, eo:eo + w],
                             start=(j == 0), stop=(j == len(chunks) - 1),
                             skip_group_check=True)
        cst = sb_cs.tile([16, 512], F32, name="cst", tag="cst")
        nch = len(chunks)
        nc.vector.tensor_copy(cst[0:nch], CS[0:nch])
        nc.gpsimd.dma_start(out=pcs[X, rho, 0:nch], in_=cst[0:nch])

    pend = []
    matrices = range(2) if KPART in ("main", "nocs", "all") else range(0)
    nrun = 0
    for X in matrices:
        tab = tabs[X]
        for rho, r in enumerate(RUNS):
            nrun += 1
            if nrun > KRUNS:
                break
            rw = RW[rho]
            base = 128 * r
            lhsT = tab[:, :, base:base + P]
            E = sb_e.tile([P, 3200], BF16, name="E",
                          tag=f"E{(X * 6 + rho) % 2}")
            racc = sb_sm.tile([P, 3], F32, name="racc",
                              tag=f"racc{(X * 6 + rho) % 2}")
            ngrp = (rw + GRP - 1) // GRP
            for gi in range(ngrp):
                glo = gi * GRP
                ghi = min(rw, glo + GRP)
                pg = ps_mm.tile([P, GRP], F32, name="pg", tag="psmm")
                pos = 0
                while pos < ghi - glo:
                    if KPACK:
                        seg = min(512 - pos % 512, ghi - glo - pos)
                        if gi == 0 and pos == 0:
                            seg = P        # diag block first
                    else:
                        seg = min(512, ghi - glo - pos)
                    st = (pos % 512) == 0 or not KPACK
                    last = (pos + seg >= ghi - glo) or \
                        ((pos + seg) % 512 == 0) or not KPACK
                    nc.tensor.matmul(
                        pg[:, pos:pos + seg],
                        lhsT=lhsT,
                        rhs=tab[:, :, base + glo + pos:base + glo + pos + seg],
                        start=st, stop=last,
                        perf_mode=mybir.MatmulPerfMode.DoubleRow,
                        skip_group_check=True)
                    pos += seg
                nc.scalar.activation(E[:, glo:ghi], pg[:, 0:ghi - glo],
                                     AF.Exp, scale=ISC,
                                     accum_out=racc[:, gi:gi + 1])
            nc.vector.reduce_sum(out=rdcols[:, X * 12 + rho:X * 12 + rho + 1],
                                 in_=racc[:, 0:ngrp], axis=AX.X)
            if KDG:
                nc.vector.tensor_mul(dg_scr, E[:, 0:P], ident)
                nc.vector.reduce_sum(
                    out=rdcols[:, X * 12 + 6 + rho:X * 12 + 7 + rho],
                    in_=dg_scr, axis=AX.X)
            if KPART != "nocs":
                pend.append((X, rho, E))
            if len(pend) > 1:
                emit_colsums(*pend.pop(0))
    while pend:
        emit_colsums(*pend.pop(0))

    # ---- BCE phase 2 ---------------------------------------------------
    # The Ln bias comes from a tile that depends on the main loop's last
    # write, pinning these Ln ops after every Exp (one act-table switch
    # instead of a per-run EXP<->LN ping-pong).
    ln_bias = sb_sm.tile([P, 1], F32)
    nc.vector.reduce_max(out=ln_bias, in_=rdcols[:, 15:17], axis=AX.X)
    nc.vector.tensor_scalar_mul(ln_bias, ln_bias, 0.0)
    nc.vector.tensor_scalar_add(ln_bias, ln_bias, 1.0)
    for i in range(1 + V):
        e, pb = bce_e[i], bce_pb[i]
        nc.scalar.activation(e, e, AF.Ln, bias=ln_bias)  # log1p(exp(-|x|))
        nc.vector.tensor_add(pb, pb, e)
        nc.vector.tensor_mul(pb, pb, msk_t)
        nc.vector.reduce_sum(out=partcols[:, i:i + 1], in_=pb, axis=AX.X)

    nc.gpsimd.dma_start(out=prd, in_=rdcols)
    nc.gpsimd.dma_start(out=pbce, in_=partcols)


# ---------------------------------------------------------------- program
def build_program():
    nc = bacc.Bacc("TRN2", target_bir_lowering=False, debug=False,
                   num_devices=NCORES)
    io = (
        nc.dram_tensor("stab8", (4, P, 2, TW // 4), FP8,
                       kind="ExternalInput").ap(),
        nc.dram_tensor("utab8", (4, P, 2, TW // 4), FP8,
                       kind="ExternalInput").ap(),
        nc.dram_tensor("slhs", (2, P, MC), BF16, kind="ExternalInput").ap(),
        nc.dram_tensor("ulhs", (2, P, MC), BF16, kind="ExternalInput").ap(),
        nc.dram_tensor("snode", (2, P, MC), BF16, kind="ExternalInput").ap(),
        nc.dram_tensor("ssel", (2, P, 1), BF16, kind="ExternalInput").ap(),
        nc.dram_tensor("blog", (P, W), F32, kind="ExternalInput").ap(),
        nc.dram_tensor("vlog", (V, P, W), F32, kind="ExternalInput").ap(),
        nc.dram_tensor("blab", (P, W), F32, kind="ExternalInput").ap(),
        nc.dram_tensor("bmsk", (P, W), F32, kind="ExternalInput").ap(),
        nc.dram_tensor("prd", (P, 24), F32, kind="ExternalOutput").ap(),
        nc.dram_tensor("pcs", (2, 6, 6, 512), F32, kind="ExternalOutput").ap(),
        nc.dram_tensor("pcor", (4, MC), F32, kind="ExternalOutput").ap(),
        nc.dram_tensor("pbce", (P, 8), F32, kind="ExternalOutput").ap(),
    )
    with tile.TileContext(nc) as tc:
        with ExitStack() as ctx:
            _loss_body(ctx, tc, io)
    nc.compile()
    return nc


def get_program():
    if "nc" not in _PROGRAM_CACHE:
        _PROGRAM_CACHE["nc"] = build_program()
    return _PROGRAM_CACHE["nc"]


# ---------------------------------------------------------------- host side
def shard_inputs(fused_logit, view_logits, proj, labels, train_mask,
                 train_pos_idx, train_neg_idx, unlabeled_idx):
    import ml_dtypes

    fused_logit = np.asarray(fused_logit, dtype=np.float32)
    view_logits = np.asarray(view_logits, dtype=np.float32)
    proj = np.asarray(proj, dtype=np.float32)
    labels = np.asarray(labels, dtype=np.float32)
    maskf = np.asarray(train_mask).astype(np.float32)

    lab_idx = np.concatenate([np.asarray(train_pos_idx),
                              np.asarray(train_neg_idx)]).astype(np.int64)
    unl_idx = np.asarray(unlabeled_idx).astype(np.int64)

    zf = proj[:, lab_idx, :].transpose(1, 0, 2).reshape(M, D)
    zu = proj[:, unl_idx, :].transpose(1, 0, 2).reshape(M, D)
    zfT8 = zf.T.astype(ml_dtypes.float8_e4m3)       # [256, 6144]
    zuT8 = zu.T.astype(ml_dtypes.float8_e4m3)

    zfT16 = zf.T.astype(ml_dtypes.bfloat16)
    zuT16 = zu.T.astype(ml_dtypes.bfloat16)
    s_lab1 = zf[:PP * V].sum(axis=0)                # [256] label-1 sum
    s_lab0 = zf[PP * V:].sum(axis=0)
    s_node = zu.reshape(U, V, D).sum(axis=1)        # [2048, 256]
    snodeT = np.repeat(s_node, V, axis=0).T.astype(ml_dtypes.bfloat16)

    def rot_tab(z8, c):
        r = np.roll(z8, -ROT * c, axis=1)
        ext = np.concatenate([r, r[:, :TW - M]], axis=1)      # [256, 6400]
        return np.ascontiguousarray(
            ext.reshape(2, P, 4, TW // 4).transpose(2, 1, 0, 3))

    def pack_bce(x):
        out = np.zeros((NCORES, P, W), dtype=np.float32)
        flat = out.reshape(NCORES, P * W)
        flat[:, :NS] = x.reshape(NCORES, NS)
        return out

    blog = pack_bce(fused_logit)
    vlog = np.stack([pack_bce(view_logits[v]) for v in range(V)], axis=1)
    blab = pack_bce(labels)
    bmsk = pack_bce(maskf)

    in_maps = []
    for c in range(NCORES):
        j0 = c * MC
        sse = (s_lab1 if c < NCORES // 2 else s_lab0).astype(
            ml_dtypes.bfloat16).reshape(2, P, 1)
        in_maps.append(dict(
            stab8=rot_tab(zfT8, c),
            utab8=rot_tab(zuT8, c),
            slhs=np.ascontiguousarray(
                zfT16[:, j0:j0 + MC]).reshape(2, P, MC),
            ulhs=np.ascontiguousarray(
                zuT16[:, j0:j0 + MC]).reshape(2, P, MC),
            snode=np.ascontiguousarray(
                snodeT[:, j0:j0 + MC]).reshape(2, P, MC),
            ssel=sse,
            blog=blog[c],
            vlog=vlog[c],
            blab=blab[c],
            bmsk=bmsk[c],
        ))
    return in_maps


def combine_partials(results):
    """results: per-core dicts with prd/pcs/pcor/pbce -> final (5,) losses."""
    den = np.zeros((2, M), dtype=np.float64)
    dgv = np.zeros((2, M), dtype=np.float64)
    for c, res in enumerate(results):
        prd = res["prd"].astype(np.float64)           # [128, 24]
        pcs = res["pcs"].astype(np.float64)           # [2, 6, 6, 512]
        for X in range(2):
            for rho, r in enumerate(RUNS):
                gt = (r + 3 * c) % NT
                rows = slice(P * gt, P * gt + P)
                den[X][rows] += prd[:, X * 12 + rho]
                dgv[X][rows] += prd[:, X * 12 + 6 + rho]
                for ci, (a, w) in enumerate(run_chunks(rho)):
                    cols = (np.arange(a, a + w) + ROT * c) % M
                    den[X][cols] += pcs[X, rho, ci, :w]
    den = den - dgv + 1e-12

    qs, ss, qu, ssu = np.concatenate(
        [r["pcor"].astype(np.float64) for r in results], axis=1)
    pt_s = (qs - ss) / (TEMP * SUP_CNT)
    pt_u = (qu - ssu) / (TEMP * (V - 1))
    sup = float(np.mean(np.log(den[0]) - pt_s))
    unsup = float(np.mean(np.log(den[1]) - pt_u))

    pb = np.stack([r["pbce"] for r in results]).astype(np.float64)
    tot = pb.sum(axis=(0, 1))                         # [8]
    cnt = max(tot[4], 1.0)
    main = tot[0] / cnt
    view = (tot[1] + tot[2] + tot[3]) / (V * cnt)
    total = L_MAIN * main + L_VIEW * view + L_SUP * sup + L_UNSUP * unsup
    return np.array([total, main, view, sup, unsup], dtype=np.float32)


def kernel(**inputs) -> np.ndarray:
    in_maps = shard_inputs(**inputs)
    nc = get_program()
    res = bass_utils.run_bass_kernel_spmd(nc, in_maps,
                                          core_ids=list(range(NCORES)))
    return combine_partials(res.results)


# revision 33
# speedup vs baseline: 1.1048x; 1.0601x over previous
"""Trainium2 Bass kernel for nn_Loss_fun_24421184045291.

Loss = BCE(fused) + mean_v BCE(view_v) + sup_contrastive + 0.2 * unsup_consistency.

Math reductions (vs the reference):
  * sup denominator mask == ~eye; pos_count == 3071 for every anchor; all
    anchors valid (these follow from the index structure, not the values).
  * positive-pair sums collapse analytically:
        sup   pt_i = (zf_i . S_label(i) - ||zf_i||^2) / (T * 3071)
        unsup pt_i = (z_i . S_node(i)  - ||z_i||^2) / (T * (V-1))
  * the unsup re-normalization is a no-op: proj rows are unit-norm, so
    zn = zf * (1 +- 1e-7); skip it.
  * both 6144x6144 similarity matrices are symmetric: each unordered block
    pair is computed once via a cyclic cover.  exp row-sums ride the scalar
    engine's accum_out; the transposed contributions are column sums of the
    exp'd blocks (ones-indicator matmuls stacking into PSUM partition rows,
    one DVE copy per run).

Cyclic symmetric cover (48 row-tiles of 128): tile ti owns blocks
(ti, (ti+d) mod 48) for d = 0..23, plus d = 24 when ti < 24.  Every unordered
pair is computed exactly once, and each tile's column window is contiguous
mod 6144.  Core c takes tiles {3c, 3c+1, 3c+2, 24+3c, 25+3c, 26+3c}.  The
host ships each core a column-rotated fp8 table (by -384c, with a 256-col
wrap extension), so the device program is identical on all cores (SPMD) and
all per-core variation is data.  Matmuls run in fp8(e4m3) DoubleRow perf
mode (K=256 per instruction, 2x PE rate).  Each core ships per-row partials
(rowsum/diag columns, colsum vectors, positive-term row-dots, BCE partial
sums); the host assembles denominators, logs, and means in float64.
"""

import sys
from contextlib import ExitStack

import numpy as np

if "/opt/trn_rl_repo" not in sys.path:
    sys.path.insert(0, "/opt/trn_rl_repo")

import concourse.bass as bass
import concourse.tile as tile
from concourse import bacc, mybir
from concourse import bass_utils
from concourse.masks import make_identity

# ---------------------------------------------------------------- constants
TEMP = 0.2
ISC = 1.0 / TEMP
L_MAIN, L_VIEW, L_SUP, L_UNSUP = 1.0, 1.0, 1.0, 0.2
N, D, V, PP, NEG, U = 100000, 256, 3, 1024, 1024, 2048

NCORES = 8
M = (PP + NEG) * V          # 6144 rows/cols of both similarity matrices
P = 128
NT = M // P                 # 48 row tiles
MC = M // NCORES            # 768 pt rows per core
NS = N // NCORES            # 12500 BCE elements per core
W = 98                      # padded BCE free width (128*98 = 12544 >= 12500)
SUP_CNT = float((PP - 1) * V + (V - 1))   # 3071 positives per sup anchor
GRP = 1536                  # psum group width (3 banks)
TW = M + 256                # rotated table width incl. wrap extension (6400)
ROT = 384                   # per-core column rotation step (3 tiles)
RUNS = (0, 1, 2, 24, 25, 26)          # local row tiles per core
RW = (3200, 3200, 3200, 3072, 3072, 3072)   # run widths (incl. 128 diag)

F32 = mybir.dt.float32
BF16 = mybir.dt.bfloat16
FP8 = mybir.dt.float8e4

_PROGRAM_CACHE = {}
import os
KPART = os.environ.get("KPART", "all")   # bce | corr | main | nocs | all
KRUNS = int(os.environ.get("KRUNS", "12"))   # number of (X, rho) runs
KDG = os.environ.get("KDG", "1") == "1"      # diag extract via custom DVE op
KPACK = os.environ.get("KPACK", "1") == "1"  # pack diag+chunks into banks


def run_chunks(rho):
    """Colsum chunks of run rho as (local_col_start, width), diag excluded."""
    base = 128 * RUNS[rho] + 128
    total = RW[rho] - 128
    out = []
    a = 0
    while a < total:
        w = min(512, total - a)
        out.append((base + a, w))
        a += w
    return out


# ---------------------------------------------------------------- device code
def _loss_body(ctx: ExitStack, tc, io):
    nc = tc.nc
    AF = mybir.ActivationFunctionType
    OP = mybir.AluOpType
    AX = mybir.AxisListType

    (stab8, utab8, slhs, ssel, blog, prd, pcs, pcor, pbce) = io

    sb_tab = ctx.enter_context(tc.tile_pool(name="sb_tab", bufs=1))
    sb_e = ctx.enter_context(tc.tile_pool(name="sb_e", bufs=2))
    sb_sl = ctx.enter_context(tc.tile_pool(name="sb_sl", bufs=1))
    sb_tmp = ctx.enter_context(tc.tile_pool(name="sb_tmp", bufs=2))
    sb_cs = ctx.enter_context(tc.tile_pool(name="sb_cs", bufs=2))
    sb_sm = ctx.enter_context(tc.tile_pool(name="sb_sm", bufs=1))
    sb_bce = ctx.enter_context(tc.tile_pool(name="sb_bce", bufs=2))
    ps_mm = ctx.enter_context(tc.tile_pool(name="ps_mm", bufs=2, space="PSUM"))
    ps_cs = ctx.enter_context(tc.tile_pool(name="ps_cs", bufs=2, space="PSUM"))

    # ---- constants -----------------------------------------------------
    ident = sb_sm.tile([P, P], F32)
    make_identity(nc, ident[:])
    indT = sb_sm.tile([P, 16], BF16)
    nc.vector.memset(indT, 0.0)
    nc.vector.memset(indT[:, 7:8], 1.0)
    ones_b = sb_sm.tile([P, 1], BF16)
    nc.vector.memset(ones_b, 1.0)

    # ---- DMA issues ----------------------------------------------------
    # Each dma_start costs ~1us of queue-side descriptor generation before
    # any data moves, so: few, large DMAs.  gpsimd queue carries the fp8
    # tables (sup first, in 3 pieces so run 0 starts early); sync carries
    # the consolidated small inputs, then outputs.
    tabs = []
    for nm in ("st", "ut"):
        tabs.append(sb_tab.tile([P, 2, TW], FP8, name=nm, tag=nm))
    cw = TW // 4

    def tab_dma(ti, glo, ghi, eng):
        src = (stab8, utab8)[ti]
        eng.dma_start(out=tabs[ti][:, :, glo * cw:ghi * cw],
                      in_=src[:, :, glo * cw:ghi * cw])

    tab_dma(0, 0, 1, nc.gpsimd)
    tab_dma(0, 1, 2, nc.gpsimd)

    # consolidated bf16 slices: [128, 3 (sl, ul, sn), 2 (k), 768]
    slab = sb_sl.tile([P, 3, 2, MC], BF16, name="slab", tag="slab")
    nc.sync.dma_start(out=slab, in_=slhs)
    sl = [slab[:, 0, k] for k in range(2)]
    ul = [slab[:, 1, k] for k in range(2)]
    sn = [slab[:, 2, k] for k in range(2)]
    sse = []
    for k in range(2):
        t = sb_sl.tile([P, 1], BF16, name=f"se{k}", tag=f"se{k}")
        nc.sync.dma_start(out=t, in_=ssel[k])
        sse.append(t)

    # consolidated BCE ball: [128, 6, 98] = blog, vlog*3, blab, bmsk
    ball = sb_bce.tile([P, 6, W], F32, name="ball", tag="ball")
    nc.sync.dma_start(out=ball, in_=blog)
    bce_x = [ball[:, i] for i in range(4)]
    lab_t = ball[:, 4]
    msk_t = ball[:, 5]

    tab_dma(0, 2, 4, nc.gpsimd)
    tab_dma(1, 0, 2, nc.gpsimd)
    tab_dma(1, 2, 4, nc.gpsimd)

    # ---- BCE phase 1 ---------------------------------------------------
    bce_e, bce_pb = [], []
    for i in range(1 + V):
        x = bce_x[i]
        e = sb_sm.tile([P, W], F32, name=f"bce_e{i}", tag=f"bce_e{i}")
        nc.scalar.activation(e, x, AF.Abs)
        nc.scalar.activation(e, e, AF.Exp, scale=-1.0)
        bce_e.append(e)
        pb = sb_sm.tile([P, W], F32, name=f"bce_pb{i}", tag=f"bce_pb{i}")
        nc.scalar.activation(pb, x, AF.Relu)
        xy = sb_bce.tile([P, W], F32, name="bce_xy", tag="bce_xy")
        nc.vector.tensor_mul(xy, x, lab_t)
        nc.vector.tensor_sub(pb, pb, xy)
        bce_pb.append(pb)

    partcols = sb_sm.tile([P, 8], F32)
    nc.vector.memset(partcols, 0.0)
    nc.vector.reduce_sum(out=partcols[:, 4:5], in_=msk_t, axis=AX.X)

    # ---- corrections: qs, ss, qu, ssu ----------------------------------
    def rowdot(vec_tiles, rhs_tiles, tag):
        res = sb_sm.tile([1, MC], F32, name=f"rd_{tag}", tag=f"rd_{tag}")
        for j0 in range(0, MC, 512):
            w = min(512, MC - j0)
            pq = ps_cs.tile([16, 512], F32, name="pq", tag="pscs")
            for k in range(2):
                nc.tensor.matmul(pq[:1, :w], lhsT=vec_tiles[k],
                                 rhs=rhs_tiles[k][:, j0:j0 + w],
                                 start=(k == 0), stop=(k == 1))
            nc.vector.tensor_copy(res[:, j0:j0 + w], pq[:1, :w])
        return res

    sq_s, sq_u, sxx = [], [], []
    for k in range(2):
        q = sb_tmp.tile([P, MC], BF16, name="sq_s", tag=f"sq_s{k}")
        nc.vector.tensor_mul(q, sl[k], sl[k])
        sq_s.append(q)
        q = sb_tmp.tile([P, MC], BF16, name="sq_u", tag=f"sq_u{k}")
        nc.vector.tensor_mul(q, ul[k], ul[k])
        sq_u.append(q)
        q = sb_tmp.tile([P, MC], BF16, name="sx", tag=f"sx{k}")
        nc.vector.tensor_mul(q, ul[k], sn[k])
        sxx.append(q)

    if KPART in ("corr", "main", "nocs", "all"):
        corr_rows = [rowdot(sse, sl, "qs"),
                     rowdot([ones_b, ones_b], sq_s, "ss"),
                     rowdot([ones_b, ones_b], sxx, "qu"),
                     rowdot([ones_b, ones_b], sq_u, "ssu")]
        for r, row in enumerate(corr_rows):
            nc.gpsimd.dma_start(out=pcor[r:r + 1], in_=row)

    # ---- main symmetric cyclic-cover loop ------------------------------
    rdcols = sb_sm.tile([P, 24], F32)
    nc.vector.memset(rdcols, 0.0)
    dg_scr = sb_tmp.tile([P, P], BF16, name="dg_scr", tag="dg_scr")

    def emit_colsums(X, rho, E):
        chunks = run_chunks(rho)
        CS = ps_cs.tile([16, 512], F32, name="cs", tag="pscs")
        for j, (a, w) in enumerate(chunks):
            eo = a - 128 * RUNS[rho]
            nc.tensor.matmul(CS[0:8, 0:w], lhsT=indT[:, 7 - j:15 - j],
                             rhs=E[:, eo:eo + w],
                             start=(j == 0), stop=(j == len(chunks) - 1),
                             skip_group_check=True)
        cst = sb_cs.tile([16, 512], F32, name="cst", tag="cst")
        nch = len(chunks)
        nc.vector.tensor_copy(cst[0:nch], CS[0:nch])
        nc.gpsimd.dma_start(out=pcs[X, rho, 0:nch], in_=cst[0:nch])

    pend = []
    matrices = range(2) if KPART in ("main", "nocs", "all") else range(0)
    nrun = 0
    for X in matrices:
        tab = tabs[X]
        for rho, r in enumerate(RUNS):
            nrun += 1
            if nrun > KRUNS:
                break
            rw = RW[rho]
            base = 128 * r
            lhsT = tab[:, :, base:base + P]
            E = sb_e.tile([P, 3200], BF16, name="E",
                          tag=f"E{(X * 6 + rho) % 2}")
            racc = sb_sm.tile([P, 3], F32, name="racc",
                              tag=f"racc{(X * 6 + rho) % 2}")
            ngrp = (rw + GRP - 1) // GRP
            for gi in range(ngrp):
                glo = gi * GRP
                ghi = min(rw, glo + GRP)
                pg = ps_mm.tile([P, GRP], F32, name="pg", tag="psmm")
                pos = 0
                while pos < ghi - glo:
                    if KPACK:
                        seg = min(512 - pos % 512, ghi - glo - pos)
                        if gi == 0 and pos == 0:
                            seg = P        # diag block first
                    else:
                        seg = min(512, ghi - glo - pos)
                    st = (pos % 512) == 0 or not KPACK
                    last = (pos + seg >= ghi - glo) or \
                        ((pos + seg) % 512 == 0) or not KPACK
                    nc.tensor.matmul(
                        pg[:, pos:pos + seg],
                        lhsT=lhsT,
                        rhs=tab[:, :, base + glo + pos:base + glo + pos + seg],
                        start=st, stop=last,
                        perf_mode=mybir.MatmulPerfMode.DoubleRow,
                        skip_group_check=True)
                    pos += seg
                nc.scalar.activation(E[:, glo:ghi], pg[:, 0:ghi - glo],
                                     AF.Exp, scale=ISC,
                                     accum_out=racc[:, gi:gi + 1])
            nc.vector.reduce_sum(out=rdcols[:, X * 12 + rho:X * 12 + rho + 1],
                                 in_=racc[:, 0:ngrp], axis=AX.X)
            if KDG:
                nc.vector.tensor_mul(dg_scr, E[:, 0:P], ident)
                nc.vector.reduce_sum(
                    out=rdcols[:, X * 12 + 6 + rho:X * 12 + 7 + rho],
                    in_=dg_scr, axis=AX.X)
            if KPART != "nocs":
                pend.append((X, rho, E))
            if len(pend) > 1:
                emit_colsums(*pend.pop(0))
    while pend:
        emit_colsums(*pend.pop(0))

    # ---- BCE phase 2 ---------------------------------------------------
    # The Ln bias comes from a tile that depends on the main loop's last
    # write, pinning these Ln ops after every Exp (one act-table switch
    # instead of a per-run EXP<->LN ping-pong).
    ln_bias = sb_sm.tile([P, 1], F32)
    nc.vector.reduce_max(out=ln_bias, in_=rdcols[:, 15:17], axis=AX.X)
    nc.vector.tensor_scalar_mul(ln_bias, ln_bias, 0.0)
    nc.vector.tensor_scalar_add(ln_bias, ln_bias, 1.0)
    for i in range(1 + V):
        e, pb = bce_e[i], bce_pb[i]
        nc.scalar.activation(e, e, AF.Ln, bias=ln_bias)  # log1p(exp(-|x|))
        nc.vector.tensor_add(pb, pb, e)
        nc.vector.tensor_mul(pb, pb, msk_t)
        nc.vector.reduce_sum(out=partcols[:, i:i + 1], in_=pb, axis=AX.X)

    nc.gpsimd.dma_start(out=prd, in_=rdcols)
    nc.gpsimd.dma_start(out=pbce, in_=partcols)


# ---------------------------------------------------------------- program
def build_program():
    nc = bacc.Bacc("TRN2", target_bir_lowering=False, debug=False,
                   num_devices=NCORES)
    io = (
        nc.dram_tensor("stab8", (P, 2, TW), FP8, kind="ExternalInput").ap(),
        nc.dram_tensor("utab8", (P, 2, TW), FP8, kind="ExternalInput").ap(),
        nc.dram_tensor("slhs", (P, 3, 2, MC), BF16, kind="ExternalInput").ap(),
        nc.dram_tensor("ssel", (2, P, 1), BF16, kind="ExternalInput").ap(),
        nc.dram_tensor("blog", (P, 6, W), F32, kind="ExternalInput").ap(),
        nc.dram_tensor("prd", (P, 24), F32, kind="ExternalOutput").ap(),
        nc.dram_tensor("pcs", (2, 6, 6, 512), F32, kind="ExternalOutput").ap(),
        nc.dram_tensor("pcor", (4, MC), F32, kind="ExternalOutput").ap(),
        nc.dram_tensor("pbce", (P, 8), F32, kind="ExternalOutput").ap(),
    )
    with tile.TileContext(nc) as tc:
        with ExitStack() as ctx:
            _loss_body(ctx, tc, io)
    nc.compile()
    return nc


def get_program():
    if "nc" not in _PROGRAM_CACHE:
        _PROGRAM_CACHE["nc"] = build_program()
    return _PROGRAM_CACHE["nc"]


# ---------------------------------------------------------------- host side
def shard_inputs(fused_logit, view_logits, proj, labels, train_mask,
                 train_pos_idx, train_neg_idx, unlabeled_idx):
    import ml_dtypes

    fused_logit = np.asarray(fused_logit, dtype=np.float32)
    view_logits = np.asarray(view_logits, dtype=np.float32)
    proj = np.asarray(proj, dtype=np.float32)
    labels = np.asarray(labels, dtype=np.float32)
    maskf = np.asarray(train_mask).astype(np.float32)

    lab_idx = np.concatenate([np.asarray(train_pos_idx),
                              np.asarray(train_neg_idx)]).astype(np.int64)
    unl_idx = np.asarray(unlabeled_idx).astype(np.int64)

    zf = proj[:, lab_idx, :].transpose(1, 0, 2).reshape(M, D)
    zu = proj[:, unl_idx, :].transpose(1, 0, 2).reshape(M, D)
    zfT8 = zf.T.astype(ml_dtypes.float8_e4m3)       # [256, 6144]
    zuT8 = zu.T.astype(ml_dtypes.float8_e4m3)

    zfT16 = zf.T.astype(ml_dtypes.bfloat16)
    zuT16 = zu.T.astype(ml_dtypes.bfloat16)
    s_lab1 = zf[:PP * V].sum(axis=0)                # [256] label-1 sum
    s_lab0 = zf[PP * V:].sum(axis=0)
    s_node = zu.reshape(U, V, D).sum(axis=1)        # [2048, 256]
    snodeT = np.repeat(s_node, V, axis=0).T.astype(ml_dtypes.bfloat16)

    def rot_tab(z8, c):
        r = np.roll(z8, -ROT * c, axis=1)
        ext = np.concatenate([r, r[:, :TW - M]], axis=1)      # [256, 6400]
        return np.ascontiguousarray(
            ext.reshape(2, P, TW).transpose(1, 0, 2))         # [128, 2, TW]

    def pack_bce(x):
        out = np.zeros((NCORES, P, W), dtype=np.float32)
        flat = out.reshape(NCORES, P * W)
        flat[:, :NS] = x.reshape(NCORES, NS)
        return out

    # BCE ball: [core, 128, 6, 98] = blog, vlog*3, blab, bmsk
    ball = np.stack([pack_bce(fused_logit)]
                    + [pack_bce(view_logits[v]) for v in range(V)]
                    + [pack_bce(labels), pack_bce(maskf)],
                    axis=2)                       # [NCORES, P, 6, W]

    in_maps = []
    for c in range(NCORES):
        j0 = c * MC
        sse = (s_lab1 if c < NCORES // 2 else s_lab0).astype(
            ml_dtypes.bfloat16).reshape(2, P, 1)
        # slab: [128, 3, 2, 768]
        slab = np.stack([
            zfT16[:, j0:j0 + MC].reshape(2, P, MC),
            zuT16[:, j0:j0 + MC].reshape(2, P, MC),
            snodeT[:, j0:j0 + MC].reshape(2, P, MC),
        ], axis=0).transpose(2, 0, 1, 3)          # [P, 3, 2, MC]
        in_maps.append(dict(
            stab8=rot_tab(zfT8, c),
            utab8=rot_tab(zuT8, c),
            slhs=np.ascontiguousarray(slab),
            ssel=sse,
            blog=np.ascontiguousarray(ball[c]),
        ))
    return in_maps


def combine_partials(results):
    """results: per-core dicts with prd/pcs/pcor/pbce -> final (5,) losses."""
    den = np.zeros((2, M), dtype=np.float64)
    dgv = np.zeros((2, M), dtype=np.float64)
    for c, res in enumerate(results):
        prd = res["prd"].astype(np.float64)           # [128, 24]
        pcs = res["pcs"].astype(np.float64)           # [2, 6, 6, 512]
        for X in range(2):
            for rho, r in enumerate(RUNS):
                gt = (r + 3 * c) % NT
                rows = slice(P * gt, P * gt + P)
                den[X][rows] += prd[:, X * 12 + rho]
                dgv[X][rows] += prd[:, X * 12 + 6 + rho]
                for ci, (a, w) in enumerate(run_chunks(rho)):
                    cols = (np.arange(a, a + w) + ROT * c) % M
                    den[X][cols] += pcs[X, rho, ci, :w]
    den = den - dgv + 1e-12

    qs, ss, qu, ssu = np.concatenate(
        [r["pcor"].astype(np.float64) for r in results], axis=1)
    pt_s = (qs - ss) / (TEMP * SUP_CNT)
    pt_u = (qu - ssu) / (TEMP * (V - 1))
    sup = float(np.mean(np.log(den[0]) - pt_s))
    unsup = float(np.mean(np.log(den[1]) - pt_u))

    pb = np.stack([r["pbce"] for r in results]).astype(np.float64)
    tot = pb.sum(axis=(0, 1))                         # [8]
    cnt = max(tot[4], 1.0)
    main = tot[0] / cnt
    view = (tot[1] + tot[2] + tot[3]) / (V * cnt)
    total = L_MAIN * main + L_VIEW * view + L_SUP * sup + L_UNSUP * unsup
    return np.array([total, main, view, sup, unsup], dtype=np.float32)


def kernel(**inputs) -> np.ndarray:
    in_maps = shard_inputs(**inputs)
    nc = get_program()
    res = bass_utils.run_bass_kernel_spmd(nc, in_maps,
                                          core_ids=list(range(NCORES)))
    return combine_partials(res.results)


# revision 34
# speedup vs baseline: 1.1153x; 1.0095x over previous
"""Trainium2 Bass kernel for nn_Loss_fun_24421184045291.

Loss = BCE(fused) + mean_v BCE(view_v) + sup_contrastive + 0.2 * unsup_consistency.

Math reductions (vs the reference):
  * sup denominator mask == ~eye; pos_count == 3071 for every anchor; all
    anchors valid (these follow from the index structure, not the values).
  * positive-pair sums collapse analytically:
        sup   pt_i = (zf_i . S_label(i) - ||zf_i||^2) / (T * 3071)
        unsup pt_i = (z_i . S_node(i)  - ||z_i||^2) / (T * (V-1))
  * the unsup re-normalization is a no-op: proj rows are unit-norm, so
    zn = zf * (1 +- 1e-7); skip it.
  * both 6144x6144 similarity matrices are symmetric: each unordered block
    pair is computed once via a cyclic cover.  exp row-sums ride the scalar
    engine's accum_out; the transposed contributions are column sums of the
    exp'd blocks (ones-indicator matmuls stacking into PSUM partition rows,
    one DVE copy per run).

Cyclic symmetric cover (48 row-tiles of 128): tile ti owns blocks
(ti, (ti+d) mod 48) for d = 0..23, plus d = 24 when ti < 24.  Every unordered
pair is computed exactly once, and each tile's column window is contiguous
mod 6144.  Core c takes tiles {3c, 3c+1, 3c+2, 24+3c, 25+3c, 26+3c}.  The
host ships each core a column-rotated fp8 table (by -384c, with a 256-col
wrap extension), so the device program is identical on all cores (SPMD) and
all per-core variation is data.  Matmuls run in fp8(e4m3) DoubleRow perf
mode (K=256 per instruction, 2x PE rate).  Each core ships per-row partials
(rowsum/diag columns, colsum vectors, positive-term row-dots, BCE partial
sums); the host assembles denominators, logs, and means in float64.
"""

import sys
from contextlib import ExitStack

import numpy as np

if "/opt/trn_rl_repo" not in sys.path:
    sys.path.insert(0, "/opt/trn_rl_repo")

import concourse.bass as bass
import concourse.tile as tile
from concourse import bacc, mybir
from concourse import bass_utils
from concourse.masks import make_identity

# ---------------------------------------------------------------- constants
TEMP = 0.2
ISC = 1.0 / TEMP
L_MAIN, L_VIEW, L_SUP, L_UNSUP = 1.0, 1.0, 1.0, 0.2
N, D, V, PP, NEG, U = 100000, 256, 3, 1024, 1024, 2048

NCORES = 8
M = (PP + NEG) * V          # 6144 rows/cols of both similarity matrices
P = 128
NT = M // P                 # 48 row tiles
MC = M // NCORES            # 768 pt rows per core
NS = N // NCORES            # 12500 BCE elements per core
W = 98                      # padded BCE free width (128*98 = 12544 >= 12500)
SUP_CNT = float((PP - 1) * V + (V - 1))   # 3071 positives per sup anchor
GRP = 1536                  # psum group width (3 banks)
TW = M + 256                # rotated table width incl. wrap extension (6400)
ROT = 384                   # per-core column rotation step (3 tiles)
RUNS = (0, 1, 2, 24, 25, 26)          # local row tiles per core
RW = (3200, 3200, 3200, 3072, 3072, 3072)   # run widths (incl. 128 diag)

F32 = mybir.dt.float32
BF16 = mybir.dt.bfloat16
FP8 = mybir.dt.float8e4

_PROGRAM_CACHE = {}
import os
KPART = os.environ.get("KPART", "all")   # bce | corr | main | nocs | all
KRUNS = int(os.environ.get("KRUNS", "12"))   # number of (X, rho) runs
KDG = os.environ.get("KDG", "1") == "1"      # diag extract via custom DVE op
KPACK = os.environ.get("KPACK", "1") == "1"  # pack diag+chunks into banks


def run_chunks(rho):
    """Colsum chunks of run rho as (local_col_start, width), diag excluded."""
    base = 128 * RUNS[rho] + 128
    total = RW[rho] - 128
    out = []
    a = 0
    while a < total:
        w = min(512, total - a)
        out.append((base + a, w))
        a += w
    return out


# ---------------------------------------------------------------- device code
def _loss_body(ctx: ExitStack, tc, io):
    nc = tc.nc
    AF = mybir.ActivationFunctionType
    OP = mybir.AluOpType
    AX = mybir.AxisListType

    (stab8, utab8, slhs, ssel, blog, prd, pcs, pcor, pbce) = io

    sb_tab = ctx.enter_context(tc.tile_pool(name="sb_tab", bufs=1))
    sb_e = ctx.enter_context(tc.tile_pool(name="sb_e", bufs=2))
    sb_sl = ctx.enter_context(tc.tile_pool(name="sb_sl", bufs=1))
    sb_tmp = ctx.enter_context(tc.tile_pool(name="sb_tmp", bufs=2))
    sb_cs = ctx.enter_context(tc.tile_pool(name="sb_cs", bufs=2))
    sb_sm = ctx.enter_context(tc.tile_pool(name="sb_sm", bufs=1))
    sb_bce = ctx.enter_context(tc.tile_pool(name="sb_bce", bufs=2))
    ps_mm = ctx.enter_context(tc.tile_pool(name="ps_mm", bufs=2, space="PSUM"))
    ps_cs = ctx.enter_context(tc.tile_pool(name="ps_cs", bufs=2, space="PSUM"))

    # ---- constants -----------------------------------------------------
    ident = sb_sm.tile([P, P], F32)
    make_identity(nc, ident[:])
    indT = sb_sm.tile([P, 16], BF16)
    nc.vector.memset(indT, 0.0)
    nc.vector.memset(indT[:, 7:8], 1.0)
    ones_b = sb_sm.tile([P, 1], BF16)
    nc.vector.memset(ones_b, 1.0)

    # ---- DMA issues ----------------------------------------------------
    # Each dma_start costs ~1us of queue-side descriptor generation before
    # any data moves, so: few, large DMAs.  gpsimd queue carries the fp8
    # tables (sup first, in 3 pieces so run 0 starts early); sync carries
    # the consolidated small inputs, then outputs.
    tabs = []
    for nm in ("st", "ut"):
        tabs.append(sb_tab.tile([P, 2, TW], FP8, name=nm, tag=nm))
    cw = TW // 4

    def tab_dma(ti, lo, hi, eng):
        src = (stab8, utab8)[ti]
        eng.dma_start(out=tabs[ti][:, :, lo:hi], in_=src[:, :, lo:hi])

    # first psum group's columns ASAP, then progressively bigger pieces
    tab_dma(0, 0, 512, nc.gpsimd)
    tab_dma(0, 512, 1664, nc.gpsimd)
    tab_dma(0, 1664, 3328, nc.gpsimd)

    # consolidated BCE ball: [128, 6, 98] = blog, vlog*3, blab, bmsk
    ball = sb_bce.tile([P, 6, W], F32, name="ball", tag="ball")
    nc.sync.dma_start(out=ball, in_=blog)
    bce_x = [ball[:, i] for i in range(4)]
    lab_t = ball[:, 4]
    msk_t = ball[:, 5]

    # consolidated bf16 slices: [128, 3 (sl, ul, sn), 2 (k), 768]
    slab = sb_sl.tile([P, 3, 2, MC], BF16, name="slab", tag="slab")
    nc.sync.dma_start(out=slab, in_=slhs)
    sl = [slab[:, 0, k] for k in range(2)]
    ul = [slab[:, 1, k] for k in range(2)]
    sn = [slab[:, 2, k] for k in range(2)]
    sse = []
    for k in range(2):
        t = sb_sl.tile([P, 1], BF16, name=f"se{k}", tag=f"se{k}")
        nc.sync.dma_start(out=t, in_=ssel[k])
        sse.append(t)

    tab_dma(0, 3328, TW, nc.gpsimd)
    tab_dma(1, 0, 3328, nc.gpsimd)
    tab_dma(1, 3328, TW, nc.gpsimd)

    # ---- BCE phase 1 ---------------------------------------------------
    bce_e, bce_pb = [], []
    for i in range(1 + V):
        x = bce_x[i]
        e = sb_sm.tile([P, W], F32, name=f"bce_e{i}", tag=f"bce_e{i}")
        nc.scalar.activation(e, x, AF.Abs)
        nc.scalar.activation(e, e, AF.Exp, scale=-1.0)
        bce_e.append(e)
        pb = sb_sm.tile([P, W], F32, name=f"bce_pb{i}", tag=f"bce_pb{i}")
        nc.scalar.activation(pb, x, AF.Relu)
        xy = sb_bce.tile([P, W], F32, name="bce_xy", tag="bce_xy")
        nc.vector.tensor_mul(xy, x, lab_t)
        nc.vector.tensor_sub(pb, pb, xy)
        bce_pb.append(pb)

    partcols = sb_sm.tile([P, 8], F32)
    nc.vector.memset(partcols, 0.0)
    nc.vector.reduce_sum(out=partcols[:, 4:5], in_=msk_t, axis=AX.X)

    # ---- corrections: qs, ss, qu, ssu ----------------------------------
    def rowdot(vec_tiles, rhs_tiles, tag):
        res = sb_sm.tile([1, MC], F32, name=f"rd_{tag}", tag=f"rd_{tag}")
        for j0 in range(0, MC, 512):
            w = min(512, MC - j0)
            pq = ps_cs.tile([16, 512], F32, name="pq", tag="pscs")
            for k in range(2):
                nc.tensor.matmul(pq[:1, :w], lhsT=vec_tiles[k],
                                 rhs=rhs_tiles[k][:, j0:j0 + w],
                                 start=(k == 0), stop=(k == 1))
            nc.vector.tensor_copy(res[:, j0:j0 + w], pq[:1, :w])
        return res

    sq_s, sq_u, sxx = [], [], []
    for k in range(2):
        q = sb_tmp.tile([P, MC], BF16, name="sq_s", tag=f"sq_s{k}")
        nc.vector.tensor_mul(q, sl[k], sl[k])
        sq_s.append(q)
        q = sb_tmp.tile([P, MC], BF16, name="sq_u", tag=f"sq_u{k}")
        nc.vector.tensor_mul(q, ul[k], ul[k])
        sq_u.append(q)
        q = sb_tmp.tile([P, MC], BF16, name="sx", tag=f"sx{k}")
        nc.vector.tensor_mul(q, ul[k], sn[k])
        sxx.append(q)

    if KPART in ("corr", "main", "nocs", "all"):
        corr_rows = [rowdot(sse, sl, "qs"),
                     rowdot([ones_b, ones_b], sq_s, "ss"),
                     rowdot([ones_b, ones_b], sxx, "qu"),
                     rowdot([ones_b, ones_b], sq_u, "ssu")]
        for r, row in enumerate(corr_rows):
            nc.gpsimd.dma_start(out=pcor[r:r + 1], in_=row)

    # ---- main symmetric cyclic-cover loop ------------------------------
    rdcols = sb_sm.tile([P, 24], F32)
    nc.vector.memset(rdcols, 0.0)
    dg_scr = sb_tmp.tile([P, P], BF16, name="dg_scr", tag="dg_scr")

    def emit_colsums(X, rho, E):
        chunks = run_chunks(rho)
        CS = ps_cs.tile([16, 512], F32, name="cs", tag="pscs")
        for j, (a, w) in enumerate(chunks):
            eo = a - 128 * RUNS[rho]
            nc.tensor.matmul(CS[0:8, 0:w], lhsT=indT[:, 7 - j:15 - j],
                             rhs=E[:, eo:eo + w],
                             start=(j == 0), stop=(j == len(chunks) - 1),
                             skip_group_check=True)
        cst = sb_cs.tile([16, 512], F32, name="cst", tag="cst")
        nch = len(chunks)
        nc.vector.tensor_copy(cst[0:nch], CS[0:nch])
        nc.gpsimd.dma_start(out=pcs[X, rho, 0:nch], in_=cst[0:nch])

    pend = []
    matrices = range(2) if KPART in ("main", "nocs", "all") else range(0)
    nrun = 0
    for X in matrices:
        tab = tabs[X]
        for rho, r in enumerate(RUNS):
            nrun += 1
            if nrun > KRUNS:
                break
            rw = RW[rho]
            base = 128 * r
            lhsT = tab[:, :, base:base + P]
            E = sb_e.tile([P, 3200], BF16, name="E",
                          tag=f"E{(X * 6 + rho) % 2}")
            racc = sb_sm.tile([P, 3], F32, name="racc",
                              tag=f"racc{(X * 6 + rho) % 2}")
            ngrp = (rw + GRP - 1) // GRP
            for gi in range(ngrp):
                glo = gi * GRP
                ghi = min(rw, glo + GRP)
                pg = ps_mm.tile([P, GRP], F32, name="pg", tag="psmm")
                pos = 0
                while pos < ghi - glo:
                    if KPACK:
                        seg = min(512 - pos % 512, ghi - glo - pos)
                        if gi == 0 and pos == 0:
                            seg = P        # diag block first
                    else:
                        seg = min(512, ghi - glo - pos)
                    st = (pos % 512) == 0 or not KPACK
                    last = (pos + seg >= ghi - glo) or \
                        ((pos + seg) % 512 == 0) or not KPACK
                    nc.tensor.matmul(
                        pg[:, pos:pos + seg],
                        lhsT=lhsT,
                        rhs=tab[:, :, base + glo + pos:base + glo + pos + seg],
                        start=st, stop=last,
                        perf_mode=mybir.MatmulPerfMode.DoubleRow,
                        skip_group_check=True)
                    pos += seg
                nc.scalar.activation(E[:, glo:ghi], pg[:, 0:ghi - glo],
                                     AF.Exp, scale=ISC,
                                     accum_out=racc[:, gi:gi + 1])
            nc.vector.reduce_sum(out=rdcols[:, X * 12 + rho:X * 12 + rho + 1],
                                 in_=racc[:, 0:ngrp], axis=AX.X)
            if KDG:
                nc.vector.tensor_mul(dg_scr, E[:, 0:P], ident)
                nc.vector.reduce_sum(
                    out=rdcols[:, X * 12 + 6 + rho:X * 12 + 7 + rho],
                    in_=dg_scr, axis=AX.X)
            if KPART != "nocs":
                pend.append((X, rho, E))
            if len(pend) > 1:
                emit_colsums(*pend.pop(0))
    while pend:
        emit_colsums(*pend.pop(0))

    # ---- BCE phase 2 ---------------------------------------------------
    # The Ln bias comes from a tile that depends on the main loop's last
    # write, pinning these Ln ops after every Exp (one act-table switch
    # instead of a per-run EXP<->LN ping-pong).
    ln_bias = sb_sm.tile([P, 1], F32)
    nc.vector.reduce_max(out=ln_bias, in_=rdcols[:, 15:17], axis=AX.X)
    nc.vector.tensor_scalar_mul(ln_bias, ln_bias, 0.0)
    nc.vector.tensor_scalar_add(ln_bias, ln_bias, 1.0)
    for i in range(1 + V):
        e, pb = bce_e[i], bce_pb[i]
        nc.scalar.activation(e, e, AF.Ln, bias=ln_bias)  # log1p(exp(-|x|))
        nc.vector.tensor_add(pb, pb, e)
        nc.vector.tensor_mul(pb, pb, msk_t)
        nc.vector.reduce_sum(out=partcols[:, i:i + 1], in_=pb, axis=AX.X)

    nc.gpsimd.dma_start(out=prd, in_=rdcols)
    nc.gpsimd.dma_start(out=pbce, in_=partcols)


# ---------------------------------------------------------------- program
def build_program():
    nc = bacc.Bacc("TRN2", target_bir_lowering=False, debug=False,
                   num_devices=NCORES)
    io = (
        nc.dram_tensor("stab8", (P, 2, TW), FP8, kind="ExternalInput").ap(),
        nc.dram_tensor("utab8", (P, 2, TW), FP8, kind="ExternalInput").ap(),
        nc.dram_tensor("slhs", (P, 3, 2, MC), BF16, kind="ExternalInput").ap(),
        nc.dram_tensor("ssel", (2, P, 1), BF16, kind="ExternalInput").ap(),
        nc.dram_tensor("blog", (P, 6, W), F32, kind="ExternalInput").ap(),
        nc.dram_tensor("prd", (P, 24), F32, kind="ExternalOutput").ap(),
        nc.dram_tensor("pcs", (2, 6, 6, 512), F32, kind="ExternalOutput").ap(),
        nc.dram_tensor("pcor", (4, MC), F32, kind="ExternalOutput").ap(),
        nc.dram_tensor("pbce", (P, 8), F32, kind="ExternalOutput").ap(),
    )
    with tile.TileContext(nc) as tc:
        with ExitStack() as ctx:
            _loss_body(ctx, tc, io)
    nc.compile()
    return nc


def get_program():
    if "nc" not in _PROGRAM_CACHE:
        _PROGRAM_CACHE["nc"] = build_program()
    return _PROGRAM_CACHE["nc"]


# ---------------------------------------------------------------- host side
def shard_inputs(fused_logit, view_logits, proj, labels, train_mask,
                 train_pos_idx, train_neg_idx, unlabeled_idx):
    import ml_dtypes

    fused_logit = np.asarray(fused_logit, dtype=np.float32)
    view_logits = np.asarray(view_logits, dtype=np.float32)
    proj = np.asarray(proj, dtype=np.float32)
    labels = np.asarray(labels, dtype=np.float32)
    maskf = np.asarray(train_mask).astype(np.float32)

    lab_idx = np.concatenate([np.asarray(train_pos_idx),
                              np.asarray(train_neg_idx)]).astype(np.int64)
    unl_idx = np.asarray(unlabeled_idx).astype(np.int64)

    zf = proj[:, lab_idx, :].transpose(1, 0, 2).reshape(M, D)
    zu = proj[:, unl_idx, :].transpose(1, 0, 2).reshape(M, D)
    zfT8 = zf.T.astype(ml_dtypes.float8_e4m3)       # [256, 6144]
    zuT8 = zu.T.astype(ml_dtypes.float8_e4m3)

    zfT16 = zf.T.astype(ml_dtypes.bfloat16)
    zuT16 = zu.T.astype(ml_dtypes.bfloat16)
    s_lab1 = zf[:PP * V].sum(axis=0)                # [256] label-1 sum
    s_lab0 = zf[PP * V:].sum(axis=0)
    s_node = zu.reshape(U, V, D).sum(axis=1)        # [2048, 256]
    snodeT = np.repeat(s_node, V, axis=0).T.astype(ml_dtypes.bfloat16)

    def rot_tab(z8, c):
        r = np.roll(z8, -ROT * c, axis=1)
        ext = np.concatenate([r, r[:, :TW - M]], axis=1)      # [256, 6400]
        return np.ascontiguousarray(
            ext.reshape(2, P, TW).transpose(1, 0, 2))         # [128, 2, TW]

    def pack_bce(x):
        out = np.zeros((NCORES, P, W), dtype=np.float32)
        flat = out.reshape(NCORES, P * W)
        flat[:, :NS] = x.reshape(NCORES, NS)
        return out

    # BCE ball: [core, 128, 6, 98] = blog, vlog*3, blab, bmsk
    ball = np.stack([pack_bce(fused_logit)]
                    + [pack_bce(view_logits[v]) for v in range(V)]
                    + [pack_bce(labels), pack_bce(maskf)],
                    axis=2)                       # [NCORES, P, 6, W]

    in_maps = []
    for c in range(NCORES):
        j0 = c * MC
        sse = (s_lab1 if c < NCORES // 2 else s_lab0).astype(
            ml_dtypes.bfloat16).reshape(2, P, 1)
        # slab: [128, 3, 2, 768]
        slab = np.stack([
            zfT16[:, j0:j0 + MC].reshape(2, P, MC),
            zuT16[:, j0:j0 + MC].reshape(2, P, MC),
            snodeT[:, j0:j0 + MC].reshape(2, P, MC),
        ], axis=0).transpose(2, 0, 1, 3)          # [P, 3, 2, MC]
        in_maps.append(dict(
            stab8=rot_tab(zfT8, c),
            utab8=rot_tab(zuT8, c),
            slhs=np.ascontiguousarray(slab),
            ssel=sse,
            blog=np.ascontiguousarray(ball[c]),
        ))
    return in_maps


def combine_partials(results):
    """results: per-core dicts with prd/pcs/pcor/pbce -> final (5,) losses."""
    den = np.zeros((2, M), dtype=np.float64)
    dgv = np.zeros((2, M), dtype=np.float64)
    for c, res in enumerate(results):
        prd = res["prd"].astype(np.float64)           # [128, 24]
        pcs = res["pcs"].astype(np.float64)           # [2, 6, 6, 512]
        for X in range(2):
            for rho, r in enumerate(RUNS):
                gt = (r + 3 * c) % NT
                rows = slice(P * gt, P * gt + P)
                den[X][rows] += prd[:, X * 12 + rho]
                dgv[X][rows] += prd[:, X * 12 + 6 + rho]
                for ci, (a, w) in enumerate(run_chunks(rho)):
                    cols = (np.arange(a, a + w) + ROT * c) % M
                    den[X][cols] += pcs[X, rho, ci, :w]
    den = den - dgv + 1e-12

    qs, ss, qu, ssu = np.concatenate(
        [r["pcor"].astype(np.float64) for r in results], axis=1)
    pt_s = (qs - ss) / (TEMP * SUP_CNT)
    pt_u = (qu - ssu) / (TEMP * (V - 1))
    sup = float(np.mean(np.log(den[0]) - pt_s))
    unsup = float(np.mean(np.log(den[1]) - pt_u))

    pb = np.stack([r["pbce"] for r in results]).astype(np.float64)
    tot = pb.sum(axis=(0, 1))                         # [8]
    cnt = max(tot[4], 1.0)
    main = tot[0] / cnt
    view = (tot[1] + tot[2] + tot[3]) / (V * cnt)
    total = L_MAIN * main + L_VIEW * view + L_SUP * sup + L_UNSUP * unsup
    return np.array([total, main, view, sup, unsup], dtype=np.float32)


def kernel(**inputs) -> np.ndarray:
    in_maps = shard_inputs(**inputs)
    nc = get_program()
    res = bass_utils.run_bass_kernel_spmd(nc, in_maps,
                                          core_ids=list(range(NCORES)))
    return combine_partials(res.results)
